# revision 53
# baseline (speedup 1.0000x reference)
"""LinearOffsetLayer Trainium2 kernel (8 NeuronCores, tensor-parallel on out_features).

Math:  A[o,i] = sum_d theta_d[d] * P_A[o,d,i] + theta0_A[o,i]
       b[o]   = theta_d @ P_b + theta0_b
       out    = input @ A.T + b                          # [4096, 1024]

Sharding: out_features (o) split 8 ways -> 128 o per core.  Each core gets its
P_A / theta0_A / P_b / theta0_b shard; input (pre-transposed on host) and
theta_d are replicated.  Each core computes out_T shard [128, 4096]; host
concatenates and transposes back.

Design (TimelineSim 79750 ns vs 264801 ns fp32 baseline, 3.3x):
  * P_A is quantized host-side to fp8 e3m4 (x16 scale; the 1/16 is folded
    into the bf16 theta sliding window).  16.8 MB/core instead of 67 MB --
    the P_A stream is the dominant DMA cost (46.6 us of the 74 us total).
    e4m3 fails the 2e-2 gate (2.26e-2); e3m4 lands at 1.18e-2.
  * x is bf16 [in_f, n]; 16 early [128,512] slices interleave into the P_A
    stream (P_A DMA outpaces the PE einsum that consumes it, so x soaks up
    the spare DMA bandwidth without delaying the P_A tail), the rest follows
    as [128, 2048] blocks.
  * einsum: theta/16 one-hot sliding window (bf16, stationary) x P_A tile
    (fp8e3, moving, 1 cycle/col) -> PSUM rows A_off[o, :]; f32 accumulate;
    one accumulation group per PSUM bank over all 128 o-rows.
  * PE warm-up: ~34 dummy matmuls while the first P_A tile is in flight burn
    the cost-model's 3us half-clock p-state ramp so the einsum runs at full
    speed; PE never idles mid-stream (idle would re-trigger the ramp).
  * A_off evicted to bf16 (DVE/ACT halves in parallel), PE-transposed via
    bf16 identity, theta0_A^T folded in during the copy-out -> aT bf16.
  * main matmul k-outer, j-outer: each 512-col out_T strip accumulates over
    k in its own PSUM bank, is evicted (DVE/ACT alternating) and written out
    in [128,1024] pair-DMAs while the PE moves to the next strip; out is
    bf16, host upcasts to f32.
Numerics (vs fp32 reference, seed-0 inputs): rel_fro 1.177e-2 (gate 2e-2).
"""

from contextlib import ExitStack

import ml_dtypes
import numpy as np

import concourse.bacc as bacc
import concourse.bass as bass
import concourse.mybir as mybir
import concourse.tile as tile
from concourse.bass_utils import run_bass_kernel_spmd
from concourse.masks import make_identity

P = 128          # partitions / d / per-core o-shard
IN_F = 1024
OUT_F = 1024
NTOK = 4096
NCORES = 8
KB = IN_F // P   # 8 k-blocks of the contraction dim
FD = 512         # psum bank free dim (f32)
NB = NTOK // FD  # 8 out n-blocks
NH = 2           # n halves for the main matmul (4 psum banks each)
XCOLS = NTOK // NH       # 2048-wide x stream blocks
F32 = mybir.dt.float32
BF16 = mybir.dt.bfloat16
FP8 = mybir.dt.float8e3

PA_G = 4                 # o-rows per P_A DMA
NPAG = P // PA_G         # 32 pa groups
PA_BUFS = 8
QSCALE = 16.0            # P_A fp8 pre-scale; 1/QSCALE folded into thwin
# early x is streamed in [128, 512] slices, one after every 2nd pa group
# starting at og=XOFF; 16 slices = (h0, k0..k3).  The rest follows the pa
# stream as full [128, 2048] blocks, with the final (h1, k7) block again in
# 512-slices so the tail matmuls/evictions pipeline with its arrival.
XOFF = 4
NEARLY_SL = 16
# PE p-state: after an idle gap the engine runs at 1/2-1/4 speed for ~3us.
# Warm it up on dummy matmuls while waiting for the first P_A tile, and keep
# it busy across the einsum->main handoff with filler matmuls.
N_WARM = 34

_CACHE = {}


def _emit_body(nc, tc, ctx, d, pools, identity):
    consts, xpool, pa_pool, asb_pool, ps_ein, ps_t, ps_main, outsb = pools

    # --- consts on the ACT DGE queue (tiny; out of the SP stream's way).
    # thwin first: the einsum's first matmul gates on it.
    thwin_sb = consts.tile([P, 2 * P - 1], BF16, name="thwin_sb")
    nc.scalar.dma_start(thwin_sb[:], d["thwin"][:, :])
    t0a_sb = consts.tile([P, IN_F], BF16, name="t0a_sb")
    nc.scalar.dma_start(t0a_sb[:], d["t0a"][:, :])
    th_sb = consts.tile([P, 1], F32, name="th_sb")
    nc.scalar.dma_start(th_sb[:], d["theta"][:, :])
    pb_sb = consts.tile([P, P], F32, name="pb_sb")
    nc.scalar.dma_start(pb_sb[:], d["pb"][:, :])
    t0b_sb = consts.tile([P, 1], F32, name="t0b_sb")
    nc.scalar.dma_start(t0b_sb[:], d["t0b"][:, :])
    b_sb = consts.tile([P, 1], F32, name="b_sb")

    # --- P_A stream + einsum, with x DMAs interleaved on the same SP queue ---
    # einsum: A_off[o, i] accumulated row-at-a-time in full-width PSUM.
    # lhsT = thwin[:, P-1-o : 2P-1-o] has theta/16 in column o, zeros elsewhere:
    # psum += lhsT.T @ pa_tile adds (theta/16).T @ (16*P_A[o]) into row o only.
    # PE warm-up: dummy matmuls on the pre-memset wu_sb tile into a scratch
    # PSUM slot (ps_main is otherwise unused until the main matmul).  The
    # cost-model PE runs at 1/2-1/4 clock for the first ~3us after going
    # busy; burning the ramp on dummies while the first P_A tile is in
    # flight keeps the whole einsum at full speed.
    wu_sb = d["wu_sb"]
    wu = ps_main.tile([P, FD], F32, name="wu", tag="po")
    for _ in range(N_WARM):
        nc.tensor.matmul(wu[:, 0:P], lhsT=wu_sb[:], rhs=wu_sb[:],
                         start=True, stop=True)

    ablk = [ps_ein.tile([P, FD], F32, name=f"ablk{h}", tag="ablk")
            for h in range(IN_F // FD)]
    x_ap = {}      # (h, k, j) -> AP for the main matmul rhs

    def emit_x_slice(h, k, j, eng=None):
        xc = xpool.tile([P, FD], BF16, name=f"xs{h}_{k}_{j}",
                        tag=f"xs{h}_{k}_{j}", bufs=1)
        (eng or nc.sync).dma_start(
            xc[:], d["xT"][k * P:(k + 1) * P,
                           h * XCOLS + j * FD:h * XCOLS + (j + 1) * FD])
        x_ap[(h, k, j)] = xc

    def emit_x_block(h, k):
        xt = xpool.tile([P, XCOLS], BF16, name=f"x{h}_{k}", tag=f"x{h}_{k}",
                        bufs=1)
        nc.sync.dma_start(
            xt[:], d["xT"][k * P:(k + 1) * P, h * XCOLS:(h + 1) * XCOLS])
        for j in range(XCOLS // FD):
            x_ap[(h, k, j)] = xt[:, j * FD:(j + 1) * FD]

    nsl = 0
    for og in range(NPAG):
        pa_t = pa_pool.tile([P, PA_G * IN_F], FP8, name="pa_t")
        nc.sync.dma_start(pa_t[:], d["pa"][og, :, :])
        if nsl < NEARLY_SL and og >= XOFF and (og - XOFF) % 2 == 0:
            emit_x_slice(0, nsl // 4, nsl % 4)
            nsl += 1
        for gi in range(PA_G):
            o = og * PA_G + gi
            for h in range(IN_F // FD):
                nc.tensor.matmul(
                    ablk[h][:, :],
                    lhsT=thwin_sb[:, P - 1 - o:2 * P - 1 - o],
                    rhs=pa_t[:, gi * IN_F + h * FD:gi * IN_F + (h + 1) * FD],
                    start=(o == 0), stop=(o == P - 1))
    while nsl < NEARLY_SL:
        emit_x_slice(0, nsl // 4, nsl % 4)
        nsl += 1
    for k in range(NEARLY_SL // 4, KB):     # rest of h0 as full blocks
        emit_x_block(0, k)
    for k in range(KB - 1):                 # h1 k0..k6 full blocks
        emit_x_block(1, k)
    for j in range(XCOLS // FD):            # h1 k7 as tail slices
        emit_x_slice(1, KB - 1, j)

    # bias: b = P_b.T @ theta + theta0_b     [o, 1]  (after einsum in PE order)
    bp = ps_t.tile([P, 1], F32, name="bp", tag="pt")
    nc.tensor.matmul(bp[:], lhsT=pb_sb[:], rhs=th_sb[:], start=True, stop=True)
    nc.vector.tensor_add(b_sb[:], bp[:], t0b_sb[:])

    # evict A_off -> bf16 with DVE/ACT halves in parallel (only DVE/ACT can
    # read PSUM), PE-transpose, then fold theta0_A^T in during the copy-out
    a_sb = asb_pool.tile([P, IN_F], BF16, name="a_sb")
    aT_sb = asb_pool.tile([P, IN_F], BF16, name="aT_sb")
    nc.vector.tensor_copy(a_sb[:, 0:FD], ablk[0][:, :])
    nc.scalar.activation(a_sb[:, FD:IN_F], ablk[1][:, :],
                         mybir.ActivationFunctionType.Identity, bias=0.0)
    pt = ps_t.tile([P, IN_F], BF16, name="pt", tag="pt")
    for k in range(KB):
        sl = slice(k * P, (k + 1) * P)
        nc.tensor.transpose(pt[:, sl], a_sb[:, sl], identity[:])
    nc.vector.tensor_add(aT_sb[:, 0:FD], pt[:, 0:FD], t0a_sb[:, 0:FD])
    nc.vector.tensor_add(aT_sb[:, FD:IN_F], pt[:, FD:IN_F],
                         t0a_sb[:, FD:IN_F])

    # --- main matmul, k-outer: out_T[:, nb] = sum_k aT[k].T @ xT[k, nb] ---
    # j-outer: each 512-col output strip finishes its k-accumulation, is
    # evicted (DVE/ACT alternating), and DMAs out (SP queue) while the PE
    # moves on -- only the last strip's eviction+DMA rides the critical tail.
    nbh = NB // NH   # 4 psum banks per half
    for h in range(NH):
        po = [ps_main.tile([P, FD], F32, name=f"po{h}_{j}", tag="po")
              for j in range(nbh)]
        ot = None
        for j in range(nbh):
            for k in range(KB):
                nc.tensor.matmul(
                    po[j][:], lhsT=aT_sb[:, k * P:(k + 1) * P],
                    rhs=x_ap[(h, k, j)],
                    start=(k == 0), stop=(k == KB - 1))
            if j % 2 == 0:
                ot = outsb.tile([P, 2 * FD], BF16, name="ot")
                nc.vector.tensor_scalar_add(ot[:, 0:FD], po[j][:],
                                            b_sb[:, 0:1])
            else:   # ACT engine: out = Identity(1.0*in + bias); then one
                nc.scalar.activation(ot[:, FD:2 * FD], po[j][:],   # pair DMA
                                     mybir.ActivationFunctionType.Identity,
                                     bias=b_sb[:, 0:1])
                n0 = (h * nbh + j - 1) * FD
                nc.sync.dma_start(d["out"][:, n0:n0 + 2 * FD], ot[:])


def _build(reps=1):
    nc = bacc.Bacc("TRN2", target_bir_lowering=False, debug=False,
                   num_devices=NCORES)

    d = {
        "xT": nc.dram_tensor("xT", [IN_F, NTOK], BF16, kind="ExternalInput"),
        "theta": nc.dram_tensor("theta", [P, 1], F32, kind="ExternalInput"),
        "pa": nc.dram_tensor("pa", [NPAG, P, PA_G * IN_F], FP8,
                             kind="ExternalInput"),
        "t0a": nc.dram_tensor("t0a", [P, IN_F], BF16, kind="ExternalInput"),
        "pb": nc.dram_tensor("pb", [P, P], F32, kind="ExternalInput"),
        "t0b": nc.dram_tensor("t0b", [P, 1], F32, kind="ExternalInput"),
        "thwin": nc.dram_tensor("thwin", [P, 2 * P - 1], BF16,
                                kind="ExternalInput"),
        "out": nc.dram_tensor("out", [P, NTOK], BF16, kind="ExternalOutput"),
    }

    with tile.TileContext(nc) as tc:
        with ExitStack() as ctx:
            pools = (
                ctx.enter_context(tc.tile_pool(name="consts", bufs=2)),
                ctx.enter_context(tc.tile_pool(name="xp", bufs=2)),
                ctx.enter_context(tc.tile_pool(name="pa", bufs=PA_BUFS)),
                ctx.enter_context(tc.tile_pool(name="asb", bufs=2)),
                ctx.enter_context(tc.tile_pool(name="ps_ein", bufs=2,
                                               space="PSUM")),
                ctx.enter_context(tc.tile_pool(name="ps_t", bufs=2,
                                               space="PSUM")),
                ctx.enter_context(tc.tile_pool(name="ps_main", bufs=4,
                                               space="PSUM")),
                ctx.enter_context(tc.tile_pool(name="outsb", bufs=8)),
            )
            const_pool = pools[0]
            wu_sb = const_pool.tile([P, P], BF16, name="wu_sb")
            nc.gpsimd.memset(wu_sb[:], 0.0)   # first: gates the PE warm-up
            d["wu_sb"] = wu_sb
            identity = const_pool.tile([P, P], BF16, name="identity")
            make_identity(nc, identity)
            for _ in range(reps):
                _emit_body(nc, tc, ctx, d, pools, identity)

    nc.compile()
    return nc


def _in_maps(inputs):
    x = np.asarray(inputs["input"], dtype=np.float32)
    theta_d = np.asarray(inputs["theta_d"], dtype=np.float32)
    theta0_A = np.asarray(inputs["theta0_A"], dtype=np.float32)
    P_A = np.asarray(inputs["P_A"], dtype=np.float32)
    theta0_b = np.asarray(inputs["theta0_b"], dtype=np.float32)
    P_b = np.asarray(inputs["P_b"], dtype=np.float32)

    xT = np.ascontiguousarray(x.T).astype(ml_dtypes.bfloat16)  # [in_f, n]
    th = np.ascontiguousarray(theta_d.reshape(P, 1))
    thwin = np.zeros((P, 2 * P - 1), ml_dtypes.bfloat16)
    thwin[:, P - 1] = (theta_d / QSCALE).astype(ml_dtypes.bfloat16)

    maps = []
    for c in range(NCORES):
        o0 = c * P
        paq = (P_A[o0:o0 + P] * QSCALE).astype(ml_dtypes.float8_e3m4)
        # [128o,128d,1024i] -> [32og, 128d, 4og_i*1024i] with 4KB dram lines
        paq = np.ascontiguousarray(
            paq.reshape(NPAG, PA_G, P, IN_F).transpose(0, 2, 1, 3)
        ).reshape(NPAG, P, PA_G * IN_F)
        # theta0_A^T packed so block kb sits at cols [kb*128, (kb+1)*128)
        t0a = np.ascontiguousarray(
            theta0_A[o0:o0 + P].T.reshape(KB, P, P).transpose(1, 0, 2)
        ).reshape(P, IN_F).astype(ml_dtypes.bfloat16)
        maps.append({
            "xT": xT,
            "theta": th,
            "pa": paq,
            "t0a": t0a,
            "pb": np.ascontiguousarray(P_b[:, o0:o0 + P]),
            "t0b": np.ascontiguousarray(theta0_b[o0:o0 + P].reshape(P, 1)),
            "thwin": thwin,
        })
    return maps


def run(inputs, trace=False):
    """Returns (output [4096,1024] f32, exec_time_ns or None)."""
    if "nc" not in _CACHE:
        _CACHE["nc"] = _build()
    nc = _CACHE["nc"]
    in_maps = _in_maps(inputs)
    for attempt in range(3):
        res = run_bass_kernel_spmd(nc, in_maps,
                                   core_ids=list(range(NCORES)), trace=trace)
        shards = [res.results[c]["out"] for c in range(NCORES)]  # [128,4096]
        outT = np.concatenate(shards, axis=0).astype(np.float32)  # [out_f, n]
        if np.isfinite(outT).all():   # guard against transient runtime flakes
            break
    return np.ascontiguousarray(outT.T), res.exec_time_ns


def kernel(**inputs):
    out, _ = run(inputs, trace=False)
    return out
